# revision 5
# baseline (speedup 1.0000x reference)
"""AtnConv (contextual attention) Trainium2 Bass kernel, 8-core SPMD.

Decomposition (per batch b, L=2304=48*48 patches, C=128):
  P  = im2col3x3(x2_pad)                    [1152, L]
  logits[p, l] = (P[:,p]*10*ma[p]) . (P[:,l]*mm[l]/max(|P[:,l]|,1e-4))
  sm = softmax over l (free dim)            [p, l]
  Yt = max(sm * post[l,p], 1e-8),  post = (1+0.5*mask_c)*mm[l]*ma[p]
  col[p, :] = Yt @ RW,  RW = im2col4x4s2(x1_pad)  [L, 2048]
  y = col2im(col)/4 ; out = concat_g relu(dilated_conv3x3(y, fuse_w[g]) + fuse_b[g])

Sharding: 8 cores = 2 batches x 4 chunks of 576 p-columns (padded to 640).
Kernel 1 (per core): Gram matmul -> softmax -> post-mul -> PE transpose ->
second matmul -> col chunk. Kernel 2 (per core = batch x row-quarter):
4 dilated fuse convs on a 40-row halo slab. Host does im2col / col2im /
scaling prep (pure indexing + tiny elementwise only).
"""
import numpy as np
import ml_dtypes
from contextlib import ExitStack

import concourse.bass as bass
import concourse.bacc as bacc
import concourse.tile as tile
import concourse.mybir as mybir
from concourse import bass_utils
from concourse.bass import ts
from concourse.masks import make_identity

BF16 = mybir.dt.bfloat16
F32 = mybir.dt.float32
H = W = 48
L = H * W           # 2304
C = 128
CHUNK = 576         # L/4 p-columns per core
CHUNKP = 640        # padded to 5*128
SCALE = 10.0
DILS = (1, 2, 4, 8)
NT = [512, 512, 512, 512, 256]   # l-dim tiling of 2304

_cache = {}


# ---------------------------------------------------------------- host prep
def _im2col3(x):
    # x [C,H,W] -> [C*9, H*W] with zero pad 1 (c-major, then ki, kj)
    Cc, Hh, Ww = x.shape
    xp = np.pad(x, ((0, 0), (1, 1), (1, 1)))
    cols = np.empty((Cc, 3, 3, Hh, Ww), np.float32)
    for ki in range(3):
        for kj in range(3):
            cols[:, ki, kj] = xp[:, ki:ki + Hh, kj:kj + Ww]
    return cols.reshape(Cc * 9, Hh * Ww)


def _im2col4s2(x):
    # x [C,96,96] -> [L, C*16], k=4 stride 2 pad 1
    Cc = x.shape[0]
    xp = np.pad(x, ((0, 0), (1, 1), (1, 1)))
    out = np.empty((H, W, Cc, 4, 4), np.float32)
    for ki in range(4):
        for kj in range(4):
            out[:, :, :, ki, kj] = xp[:, ki:ki + 2 * H:2, kj:kj + 2 * W:2].transpose(1, 2, 0)
    return out.reshape(L, Cc * 16)


def _neighbor_mask():
    M = np.zeros((L, L), np.float32)
    p = np.arange(L)
    pi, pj = p // W, p % W
    for off, sel in ((-1, pj >= 1), (1, pj <= W - 2), (W, pi <= H - 2), (-W, pi >= 1)):
        M[p[sel] + off, p[sel]] = 1.0
    return M


def _col2im(col):
    # col [L, C*16] -> [C, 96, 96] scatter-add (stride 2, pad 1)
    colr = col.reshape(H, W, C, 4, 4)
    out = np.zeros((C, 99, 99), np.float32)
    for ki in range(4):
        for kj in range(4):
            out[:, ki:ki + 96:2, kj:kj + 96:2] += colr[:, :, :, ki, kj].transpose(2, 0, 1)
    return out[:, 1:97, 1:97]


def _pack_part(a, p):
    # [N, F] -> [p, N//p, F] partition-major packing (row r = t*p + pp)
    n, f = a.shape
    return np.ascontiguousarray(a.reshape(n // p, p, f).transpose(1, 0, 2))


# ---------------------------------------------------------------- kernels
def _build_main():
    nc = bacc.Bacc("TRN2", target_bir_lowering=False, debug=False, num_devices=8)
    lhs = nc.dram_tensor("lhsP", [128, 9, CHUNKP], BF16, kind="ExternalInput").ap()
    rhs = nc.dram_tensor("rhsP", [128, 9, L], BF16, kind="ExternalInput").ap()
    post = nc.dram_tensor("post", [128, 5, L], BF16, kind="ExternalInput").ap()
    rw = nc.dram_tensor("rw", [128, 18, 2048], BF16, kind="ExternalInput").ap()
    col = nc.dram_tensor("col", [128, 5, 2048], F32, kind="ExternalOutput").ap()

    with tile.TileContext(nc) as tc, ExitStack() as ctx:
        const = ctx.enter_context(tc.tile_pool(name="const", bufs=1))
        ident = const.tile([128, 128], BF16)
        make_identity(nc, ident)
        ins = ctx.enter_context(tc.tile_pool(name="ins", bufs=1))
        s_lhs = ins.tile([128, 9, CHUNKP], BF16, tag="lhs")
        s_rhs = ins.tile([128, 9, L], BF16, tag="rhs")
        s_post = ins.tile([128, 5, L], BF16, tag="post")
        s_rw = ins.tile([128, 18, 2048], BF16, tag="rw")
        # split DMAs so the first matmuls' deps land early
        off = 0
        for sz in NT:
            nc.sync.dma_start(s_rhs[:, :, off:off + sz], rhs[:, :, off:off + sz])
            off += sz
        for m in range(5):
            nc.sync.dma_start(s_lhs[:, :, ts(m, 128)], lhs[:, :, ts(m, 128)])
        for m in range(5):
            nc.sync.dma_start(s_post[:, m, :], post[:, m, :])
        for nn in range(4):
            nc.sync.dma_start(s_rw[:, :, ts(nn, 512)], rw[:, :, ts(nn, 512)])

        # PSUM banks: Ysl 5 + P2 1 + pT 2 = 8
        psum1 = ctx.enter_context(tc.tile_pool(name="psum1", bufs=5, space="PSUM"))
        psum2 = ctx.enter_context(tc.tile_pool(name="psum2", bufs=2, space="PSUM"))
        psumT = ctx.enter_context(tc.tile_pool(name="psumT", bufs=1, space="PSUM"))
        work = ctx.enter_context(tc.tile_pool(name="work", bufs=2))
        stats = ctx.enter_context(tc.tile_pool(name="stats", bufs=3))

        def mm1_stage(m):
            ysl = []
            mx = stats.tile([128, 5], F32, tag="mx")
            for n, sz in enumerate(NT):
                off = n * 512
                Y = psum1.tile([128, 512], F32, tag="Y")
                for k in range(9):
                    nc.tensor.matmul(
                        Y[:, :sz],
                        lhsT=s_lhs[:, k, ts(m, 128)],
                        rhs=s_rhs[:, k, off:off + sz],
                        start=(k == 0), stop=(k == 8),
                    )
                # eager per-slice max (DVE runs under following matmuls)
                nc.vector.reduce_max(mx[:, n:n + 1], Y[:, :sz],
                                     axis=mybir.AxisListType.X)
                ysl.append(Y)
            return ysl, mx

        def rest_stage(m, ysl, mx):
            negm = stats.tile([128, 1], F32, tag="negm")
            nc.vector.tensor_reduce(negm, mx, axis=mybir.AxisListType.X,
                                    op=mybir.AluOpType.max, negate=True)
            sums = stats.tile([128, 5], F32, tag="sums")
            yt1 = work.tile([128, L], BF16, tag="yt1")
            for n, sz in enumerate(NT):
                off = n * 512
                ye = work.tile([128, 512], BF16, tag="ye")
                nc.scalar.activation(ye[:, :sz], ysl[n][:, :sz],
                                     mybir.ActivationFunctionType.Exp,
                                     bias=negm, accum_out=sums[:, n:n + 1])
                nc.vector.tensor_mul(yt1[:, off:off + sz], ye[:, :sz],
                                     s_post[:, m, off:off + sz])
            stot = stats.tile([128, 1], F32, tag="stot")
            nc.vector.reduce_sum(stot, sums, axis=mybir.AxisListType.X)
            rcp = stats.tile([128, 1], F32, tag="rcp")
            nc.vector.reciprocal(rcp, stot)
            yt = work.tile([128, L], BF16, tag="yt")
            nc.vector.tensor_scalar(yt, yt1, scalar1=rcp, scalar2=1e-8,
                                    op0=mybir.AluOpType.mult,
                                    op1=mybir.AluOpType.max)
            # transpose 18 [128,128] blocks: yt [p, l] -> ytT [l, p]
            # batch 4 transposes per PSUM tile -> one evict copy each
            ytT = work.tile([128, 18, 128], BF16, tag="ytT")
            for t0 in range(0, 18, 4):
                nb = min(4, 18 - t0)
                pT = psumT.tile([128, 4, 128], BF16, tag="pT")
                for k in range(t0, t0 + nb):
                    nc.tensor.transpose(pT[:, k - t0, :], yt[:, ts(k, 128)], ident)
                nc.any.tensor_copy(ytT[:, t0:t0 + nb, :], pT[:, :nb, :])
            # second matmul: col[p, co] = sum_l Yt[l, p] * RW[l, co]
            colm = work.tile([128, 2048], F32, tag="colm")
            for nn in range(4):
                P2 = psum2.tile([128, 512], F32, tag="P2")
                for k in range(18):
                    nc.tensor.matmul(P2, lhsT=ytT[:, k, :],
                                     rhs=s_rw[:, k, ts(nn, 512)],
                                     start=(k == 0), stop=(k == 17))
                nc.any.tensor_copy(colm[:, ts(nn, 512)], P2)
            nc.sync.dma_start(col[:, m, :], colm)

        # software pipeline: mm1(m+1) overlaps softmax/transpose/mm2 of m
        prev = None
        for m in range(5):
            cur = mm1_stage(m)
            if prev is not None:
                rest_stage(m - 1, *prev)
            prev = cur
        rest_stage(4, *prev)
    nc.compile()
    return nc


def _build_fuse():
    nc = bacc.Bacc("TRN2", target_bir_lowering=False, debug=False, num_devices=8)
    y = nc.dram_tensor("yslab", [128, 40, 112], BF16, kind="ExternalInput").ap()
    fw = nc.dram_tensor("fw", [128, 4, 9, 16], BF16, kind="ExternalInput").ap()
    fb = nc.dram_tensor("fb", [16, 4], F32, kind="ExternalInput").ap()
    fo = nc.dram_tensor("fo", [16, 4, 24 * 96], F32, kind="ExternalOutput").ap()

    RT = [(0, 5), (5, 5), (10, 5), (15, 5), (20, 4)]
    with tile.TileContext(nc) as tc, ExitStack() as ctx:
        ins = ctx.enter_context(tc.tile_pool(name="ins", bufs=1))
        s_y = ins.tile([128, 40, 112], BF16, tag="y")
        nc.sync.dma_start(s_y, y)
        s_w = ins.tile([128, 4, 9, 16], BF16, tag="w")
        nc.sync.dma_start(s_w, fw)
        s_b = ins.tile([16, 4], F32, tag="b")
        nc.sync.dma_start(s_b, fb)
        psum = ctx.enter_context(tc.tile_pool(name="psum", bufs=8, space="PSUM"))
        work = ctx.enter_context(tc.tile_pool(name="work", bufs=8))
        for g in range(4):
            d = DILS[g]
            for r0, nr in RT:
                ps = psum.tile([16, 512], F32, tag="ps")
                n = nr * 96
                first = True
                for ki in range(3):
                    for kj in range(3):
                        u0 = 8 + r0 + d * (ki - 1)
                        v0 = 8 + d * (kj - 1)
                        nc.tensor.matmul(
                            ps[:, :n],
                            lhsT=s_w[:, g, ki * 3 + kj, :],
                            rhs=s_y[:, u0:u0 + nr, v0:v0 + 96],
                            start=first, stop=(ki == 2 and kj == 2),
                        )
                        first = False
                ob = work.tile([16, 512], F32, tag="ob")
                nc.scalar.activation(ob[:, :n], ps[:, :n],
                                     mybir.ActivationFunctionType.Relu,
                                     bias=s_b[:, g:g + 1])
                nc.sync.dma_start(fo[:, g, r0 * 96:r0 * 96 + n], ob[:, :n])
    nc.compile()
    return nc


def _get(name, builder):
    if name not in _cache:
        _cache[name] = builder()
    return _cache[name]


# ---------------------------------------------------------------- entry
def kernel(x1, x2, mask, mask_all, fuse_w, fuse_b, _collect=None):
    x1 = np.asarray(x1, np.float32)
    x2 = np.asarray(x2, np.float32)
    mask = np.asarray(mask, np.float32)
    mask_all = np.asarray(mask_all, np.float32)
    fuse_w = np.asarray(fuse_w, np.float32)
    fuse_b = np.asarray(fuse_b, np.float32)
    N = x1.shape[0]
    bf = ml_dtypes.bfloat16

    NB = _neighbor_mask()  # [L, L]
    in_maps = []
    for b in range(N):
        P = _im2col3(x2[b])                       # [1152, L]
        norms = np.sqrt((P * P).sum(0))
        mp = _im2col3(mask[b])                    # [9, L]
        mm = (mp.mean(0) == 0.0).astype(np.float32)
        ma = mask_all[b, 0].reshape(L)
        lhs_full = P * (SCALE * ma)[None, :]      # scale col p
        rhs_full = P * (mm / np.maximum(norms, 1e-4))[None, :]
        rhs_r = _pack_part(rhs_full, 128).astype(bf)      # [128, 9, L]
        RW = _im2col4s2(x1[b])                    # [L, 2048]
        rw_r = _pack_part(RW, 128).astype(bf)             # [128, 18, 2048]
        postF = (1.0 + 0.5 * NB) * mm[:, None] * ma[None, :]  # [l, p]
        for j in range(4):
            sl = slice(j * CHUNK, (j + 1) * CHUNK)
            lhs_c = np.zeros((1152, CHUNKP), np.float32)
            lhs_c[:, :CHUNK] = lhs_full[:, sl]
            post_c = np.zeros((CHUNKP, L), np.float32)
            post_c[:CHUNK] = postF.T[sl]          # [p, l]
            in_maps.append({
                "lhsP": _pack_part(lhs_c, 128).astype(bf),
                "rhsP": rhs_r,
                "post": _pack_part(post_c, 128).astype(bf),
                "rw": rw_r,
            })
    nc1 = _get("main", _build_main)
    res1 = bass_utils.run_bass_kernel_spmd(nc1, in_maps, core_ids=list(range(8)))
    if _collect is not None:
        _collect.append(res1)

    ys = []
    for b in range(N):
        cols = []
        for j in range(4):
            r = res1.results[b * 4 + j]["col"]     # [128, 5, 2048]
            cols.append(r.transpose(1, 0, 2).reshape(CHUNKP, 2048)[:CHUNK])
        col = np.concatenate(cols, 0)              # [L, 2048]
        ys.append(_col2im(col) / 4.0)
    y = np.stack(ys)                               # [N, 128, 96, 96]

    fw_r = np.ascontiguousarray(
        fuse_w.transpose(2, 0, 3, 4, 1).reshape(128, 4, 9, 16)).astype(bf)
    fb_r = np.ascontiguousarray(fuse_b.T).astype(np.float32)  # [16, 4]
    in_maps2 = []
    for b in range(N):
        yp = np.pad(y[b], ((0, 0), (8, 8), (8, 8))).astype(bf)  # [128,112,112]
        for q in range(4):
            in_maps2.append({
                "yslab": np.ascontiguousarray(yp[:, 24 * q:24 * q + 40, :]),
                "fw": fw_r, "fb": fb_r,
            })
    nc2 = _get("fuse", _build_fuse)
    res2 = bass_utils.run_bass_kernel_spmd(nc2, in_maps2, core_ids=list(range(8)))
    if _collect is not None:
        _collect.append(res2)

    out = np.empty((N, 64, 96, 96), np.float32)
    for b in range(N):
        for q in range(4):
            r = res2.results[b * 4 + q]            # [16, 4, 2304]
            o = r["fo"].reshape(16, 4, 24, 96)
            out[b, :, 24 * q:24 * q + 24, :] = o.transpose(1, 0, 2, 3).reshape(64, 24, 96)
    return out


# revision 7
# speedup vs baseline: 1.0506x; 1.0506x over previous
"""AtnConv (contextual attention) Trainium2 Bass kernel, 8-core SPMD.

Decomposition (per batch b, L=2304=48*48 patches, C=128):
  P  = im2col3x3(x2_pad)                    [1152, L]
  logits[p, l] = (P[:,p]*10*ma[p]) . (P[:,l]*mm[l]/max(|P[:,l]|,1e-4))
  sm = softmax over l (free dim)            [p, l]
  Yt = max(sm * post[l,p], 1e-8),  post = (1+0.5*mask_c)*mm[l]*ma[p]
  col[p, :] = Yt @ RW,  RW = im2col4x4s2(x1_pad)  [L, 2048]
  y = col2im(col)/4 ; out = concat_g relu(dilated_conv3x3(y, fuse_w[g]) + fuse_b[g])

Sharding: 8 cores = 2 batches x 4 chunks of 576 p-columns (padded to 640).
Kernel 1 (per core): Gram matmul -> softmax -> post-mul -> PE transpose ->
second matmul -> col chunk. Kernel 2 (per core = batch x row-quarter):
4 dilated fuse convs on a 40-row halo slab. Host does im2col / col2im /
scaling prep (pure indexing + tiny elementwise only).
"""
import numpy as np
import ml_dtypes
from contextlib import ExitStack

import concourse.bass as bass
import concourse.bacc as bacc
import concourse.tile as tile
import concourse.mybir as mybir
from concourse import bass_utils
from concourse.bass import ts
from concourse.masks import make_identity

BF16 = mybir.dt.bfloat16
F32 = mybir.dt.float32
H = W = 48
L = H * W           # 2304
C = 128
CHUNK = 576         # L/4 p-columns per core
CHUNKP = 640        # padded to 5*128
SCALE = 10.0
DILS = (1, 2, 4, 8)
NT = [512, 512, 512, 512, 256]   # l-dim tiling of 2304

_cache = {}


# ---------------------------------------------------------------- host prep
def _im2col3(x):
    # x [C,H,W] -> [C*9, H*W] with zero pad 1 (c-major, then ki, kj)
    Cc, Hh, Ww = x.shape
    xp = np.pad(x, ((0, 0), (1, 1), (1, 1)))
    cols = np.empty((Cc, 3, 3, Hh, Ww), np.float32)
    for ki in range(3):
        for kj in range(3):
            cols[:, ki, kj] = xp[:, ki:ki + Hh, kj:kj + Ww]
    return cols.reshape(Cc * 9, Hh * Ww)


def _im2col4s2(x):
    # x [C,96,96] -> [L, C*16], k=4 stride 2 pad 1
    Cc = x.shape[0]
    xp = np.pad(x, ((0, 0), (1, 1), (1, 1)))
    out = np.empty((H, W, Cc, 4, 4), np.float32)
    for ki in range(4):
        for kj in range(4):
            out[:, :, :, ki, kj] = xp[:, ki:ki + 2 * H:2, kj:kj + 2 * W:2].transpose(1, 2, 0)
    return out.reshape(L, Cc * 16)


def _neighbor_mask():
    M = np.zeros((L, L), np.float32)
    p = np.arange(L)
    pi, pj = p // W, p % W
    for off, sel in ((-1, pj >= 1), (1, pj <= W - 2), (W, pi <= H - 2), (-W, pi >= 1)):
        M[p[sel] + off, p[sel]] = 1.0
    return M


def _col2im(col):
    # col [L, C*16] -> [C, 96, 96] scatter-add (stride 2, pad 1)
    colr = col.reshape(H, W, C, 4, 4)
    out = np.zeros((C, 99, 99), np.float32)
    for ki in range(4):
        for kj in range(4):
            out[:, ki:ki + 96:2, kj:kj + 96:2] += colr[:, :, :, ki, kj].transpose(2, 0, 1)
    return out[:, 1:97, 1:97]


def _pack_part(a, p):
    # [N, F] -> [p, N//p, F] partition-major packing (row r = t*p + pp)
    n, f = a.shape
    return np.ascontiguousarray(a.reshape(n // p, p, f).transpose(1, 0, 2))


# ---------------------------------------------------------------- kernels
def _build_main():
    nc = bacc.Bacc("TRN2", target_bir_lowering=False, debug=False, num_devices=8)
    lhs = nc.dram_tensor("lhsP", [128, 9, CHUNKP], BF16, kind="ExternalInput").ap()
    rhs = nc.dram_tensor("rhsP", [128, 9, L], BF16, kind="ExternalInput").ap()
    post = nc.dram_tensor("post", [128, 5, L], BF16, kind="ExternalInput").ap()
    rw = nc.dram_tensor("rw", [128, 18, 2048], BF16, kind="ExternalInput").ap()
    col = nc.dram_tensor("col", [128, 5, 2048], F32, kind="ExternalOutput").ap()

    with tile.TileContext(nc) as tc, ExitStack() as ctx:
        const = ctx.enter_context(tc.tile_pool(name="const", bufs=1))
        ident = const.tile([128, 128], BF16)
        make_identity(nc, ident)
        ins = ctx.enter_context(tc.tile_pool(name="ins", bufs=1))
        s_lhs = ins.tile([128, 9, CHUNKP], BF16, tag="lhs")
        s_rhs = ins.tile([128, 9, L], BF16, tag="rhs")
        s_post = ins.tile([128, 5, L], BF16, tag="post")
        s_rw = ins.tile([128, 18, 2048], BF16, tag="rw")
        # split DMAs so the first matmuls' deps land early (lhs m=0 first)
        for m in range(5):
            nc.sync.dma_start(s_lhs[:, :, ts(m, 128)], lhs[:, :, ts(m, 128)])
        off = 0
        for sz in NT:
            nc.sync.dma_start(s_rhs[:, :, off:off + sz], rhs[:, :, off:off + sz])
            off += sz
        for m in range(5):
            nc.sync.dma_start(s_post[:, m, :], post[:, m, :])
        for nn in range(4):
            nc.sync.dma_start(s_rw[:, :, ts(nn, 512)], rw[:, :, ts(nn, 512)])

        # PSUM banks: Ysl 5 + P2 1 + pT 2 = 8
        psum1 = ctx.enter_context(tc.tile_pool(name="psum1", bufs=5, space="PSUM"))
        psum2 = ctx.enter_context(tc.tile_pool(name="psum2", bufs=2, space="PSUM"))
        psumT = ctx.enter_context(tc.tile_pool(name="psumT", bufs=1, space="PSUM"))
        work = ctx.enter_context(tc.tile_pool(name="work", bufs=2))
        stats = ctx.enter_context(tc.tile_pool(name="stats", bufs=3))

        def mm1_stage(m):
            ysl = []
            mx = stats.tile([128, 5], F32, tag="mx")
            for n, sz in enumerate(NT):
                off = n * 512
                Y = psum1.tile([128, 512], F32, tag="Y")
                for k in range(9):
                    nc.tensor.matmul(
                        Y[:, :sz],
                        lhsT=s_lhs[:, k, ts(m, 128)],
                        rhs=s_rhs[:, k, off:off + sz],
                        start=(k == 0), stop=(k == 8),
                    )
                # eager per-slice max (DVE runs under following matmuls)
                nc.vector.reduce_max(mx[:, n:n + 1], Y[:, :sz],
                                     axis=mybir.AxisListType.X)
                ysl.append(Y)
            return ysl, mx

        def rest_stage(m, ysl, mx):
            negm = stats.tile([128, 1], F32, tag="negm")
            nc.vector.tensor_reduce(negm, mx, axis=mybir.AxisListType.X,
                                    op=mybir.AluOpType.max, negate=True)
            sums = stats.tile([128, 5], F32, tag="sums")
            yt1 = work.tile([128, L], BF16, tag="yt1")
            for n, sz in enumerate(NT):
                off = n * 512
                ye = work.tile([128, 512], BF16, tag="ye")
                nc.scalar.activation(ye[:, :sz], ysl[n][:, :sz],
                                     mybir.ActivationFunctionType.Exp,
                                     bias=negm, accum_out=sums[:, n:n + 1])
                nc.vector.tensor_mul(yt1[:, off:off + sz], ye[:, :sz],
                                     s_post[:, m, off:off + sz])
            stot = stats.tile([128, 1], F32, tag="stot")
            nc.vector.reduce_sum(stot, sums, axis=mybir.AxisListType.X)
            rcp = stats.tile([128, 1], F32, tag="rcp")
            nc.vector.reciprocal(rcp, stot)
            yt = work.tile([128, L], BF16, tag="yt")
            nc.vector.tensor_scalar(yt, yt1, scalar1=rcp, scalar2=1e-8,
                                    op0=mybir.AluOpType.mult,
                                    op1=mybir.AluOpType.max)
            # transpose 18 [128,128] blocks: yt [p, l] -> ytT [l, p]
            # batch 4 transposes per PSUM tile -> one evict copy each
            ytT = work.tile([128, 18, 128], BF16, tag="ytT")
            for t0 in range(0, 18, 4):
                nb = min(4, 18 - t0)
                pT = psumT.tile([128, 4, 128], BF16, tag="pT")
                for k in range(t0, t0 + nb):
                    nc.tensor.transpose(pT[:, k - t0, :], yt[:, ts(k, 128)], ident)
                nc.any.tensor_copy(ytT[:, t0:t0 + nb, :], pT[:, :nb, :])
            # second matmul: col[p, co] = sum_l Yt[l, p] * RW[l, co]
            colm = work.tile([128, 2048], F32, tag="colm")
            for nn in range(4):
                P2 = psum2.tile([128, 512], F32, tag="P2")
                for k in range(18):
                    nc.tensor.matmul(P2, lhsT=ytT[:, k, :],
                                     rhs=s_rw[:, k, ts(nn, 512)],
                                     start=(k == 0), stop=(k == 17))
                nc.any.tensor_copy(colm[:, ts(nn, 512)], P2)
            nc.sync.dma_start(col[:, m, :], colm)

        # software pipeline: mm1(m+1) overlaps softmax/transpose/mm2 of m
        prev = None
        for m in range(5):
            cur = mm1_stage(m)
            if prev is not None:
                rest_stage(m - 1, *prev)
            prev = cur
        rest_stage(4, *prev)
    nc.compile()
    return nc


def _build_fuse():
    nc = bacc.Bacc("TRN2", target_bir_lowering=False, debug=False, num_devices=8)
    y = nc.dram_tensor("yslab", [128, 40, 112], BF16, kind="ExternalInput").ap()
    fw = nc.dram_tensor("fw", [128, 4, 9, 16], BF16, kind="ExternalInput").ap()
    fb = nc.dram_tensor("fb", [16, 4], F32, kind="ExternalInput").ap()
    fo = nc.dram_tensor("fo", [16, 4, 24 * 96], F32, kind="ExternalOutput").ap()

    RT = [(0, 5), (5, 5), (10, 5), (15, 5), (20, 4)]
    with tile.TileContext(nc) as tc, ExitStack() as ctx:
        ins = ctx.enter_context(tc.tile_pool(name="ins", bufs=1))
        s_w = ins.tile([128, 4, 9, 16], BF16, tag="w")
        nc.sync.dma_start(s_w, fw)
        s_b = ins.tile([16, 4], F32, tag="b")
        nc.sync.dma_start(s_b, fb)
        s_y = ins.tile([128, 40, 112], BF16, tag="y")
        for rc in range(5):
            nc.sync.dma_start(s_y[:, 8 * rc:8 * rc + 8, :], y[:, 8 * rc:8 * rc + 8, :])
        psum = ctx.enter_context(tc.tile_pool(name="psum", bufs=8, space="PSUM"))
        work = ctx.enter_context(tc.tile_pool(name="work", bufs=8))
        for g in range(4):
            d = DILS[g]
            for r0, nr in RT:
                ps = psum.tile([16, 512], F32, tag="ps")
                n = nr * 96
                first = True
                for ki in range(3):
                    for kj in range(3):
                        u0 = 8 + r0 + d * (ki - 1)
                        v0 = 8 + d * (kj - 1)
                        nc.tensor.matmul(
                            ps[:, :n],
                            lhsT=s_w[:, g, ki * 3 + kj, :],
                            rhs=s_y[:, u0:u0 + nr, v0:v0 + 96],
                            start=first, stop=(ki == 2 and kj == 2),
                        )
                        first = False
                ob = work.tile([16, 512], F32, tag="ob")
                nc.scalar.activation(ob[:, :n], ps[:, :n],
                                     mybir.ActivationFunctionType.Relu,
                                     bias=s_b[:, g:g + 1])
                nc.sync.dma_start(fo[:, g, r0 * 96:r0 * 96 + n], ob[:, :n])
    nc.compile()
    return nc


def _get(name, builder):
    if name not in _cache:
        _cache[name] = builder()
    return _cache[name]


# ---------------------------------------------------------------- entry
def kernel(x1, x2, mask, mask_all, fuse_w, fuse_b, _collect=None):
    x1 = np.asarray(x1, np.float32)
    x2 = np.asarray(x2, np.float32)
    mask = np.asarray(mask, np.float32)
    mask_all = np.asarray(mask_all, np.float32)
    fuse_w = np.asarray(fuse_w, np.float32)
    fuse_b = np.asarray(fuse_b, np.float32)
    N = x1.shape[0]
    bf = ml_dtypes.bfloat16

    NB = _neighbor_mask()  # [L, L]
    in_maps = []
    for b in range(N):
        P = _im2col3(x2[b])                       # [1152, L]
        norms = np.sqrt((P * P).sum(0))
        mp = _im2col3(mask[b])                    # [9, L]
        mm = (mp.mean(0) == 0.0).astype(np.float32)
        ma = mask_all[b, 0].reshape(L)
        lhs_full = P * (SCALE * ma)[None, :]      # scale col p
        rhs_full = P * (mm / np.maximum(norms, 1e-4))[None, :]
        rhs_r = _pack_part(rhs_full, 128).astype(bf)      # [128, 9, L]
        RW = _im2col4s2(x1[b])                    # [L, 2048]
        rw_r = _pack_part(RW, 128).astype(bf)             # [128, 18, 2048]
        postF = (1.0 + 0.5 * NB) * mm[:, None] * ma[None, :]  # [l, p]
        for j in range(4):
            sl = slice(j * CHUNK, (j + 1) * CHUNK)
            lhs_c = np.zeros((1152, CHUNKP), np.float32)
            lhs_c[:, :CHUNK] = lhs_full[:, sl]
            post_c = np.zeros((CHUNKP, L), np.float32)
            post_c[:CHUNK] = postF.T[sl]          # [p, l]
            in_maps.append({
                "lhsP": _pack_part(lhs_c, 128).astype(bf),
                "rhsP": rhs_r,
                "post": _pack_part(post_c, 128).astype(bf),
                "rw": rw_r,
            })
    nc1 = _get("main", _build_main)
    res1 = bass_utils.run_bass_kernel_spmd(nc1, in_maps, core_ids=list(range(8)))
    if _collect is not None:
        _collect.append(res1)

    ys = []
    for b in range(N):
        cols = []
        for j in range(4):
            r = res1.results[b * 4 + j]["col"]     # [128, 5, 2048]
            cols.append(r.transpose(1, 0, 2).reshape(CHUNKP, 2048)[:CHUNK])
        col = np.concatenate(cols, 0)              # [L, 2048]
        ys.append(_col2im(col) / 4.0)
    y = np.stack(ys)                               # [N, 128, 96, 96]

    fw_r = np.ascontiguousarray(
        fuse_w.transpose(2, 0, 3, 4, 1).reshape(128, 4, 9, 16)).astype(bf)
    fb_r = np.ascontiguousarray(fuse_b.T).astype(np.float32)  # [16, 4]
    in_maps2 = []
    for b in range(N):
        yp = np.pad(y[b], ((0, 0), (8, 8), (8, 8))).astype(bf)  # [128,112,112]
        for q in range(4):
            in_maps2.append({
                "yslab": np.ascontiguousarray(yp[:, 24 * q:24 * q + 40, :]),
                "fw": fw_r, "fb": fb_r,
            })
    nc2 = _get("fuse", _build_fuse)
    res2 = bass_utils.run_bass_kernel_spmd(nc2, in_maps2, core_ids=list(range(8)))
    if _collect is not None:
        _collect.append(res2)

    out = np.empty((N, 64, 96, 96), np.float32)
    for b in range(N):
        for q in range(4):
            r = res2.results[b * 4 + q]            # [16, 4, 2304]
            o = r["fo"].reshape(16, 4, 24, 96)
            out[b, :, 24 * q:24 * q + 24, :] = o.transpose(1, 0, 2, 3).reshape(64, 24, 96)
    return out


# revision 9
# speedup vs baseline: 1.0709x; 1.0193x over previous
"""AtnConv (contextual attention) Trainium2 Bass kernel, 8-core SPMD.

Decomposition (per batch b, L=2304=48*48 patches, C=128):
  P  = im2col3x3(x2_pad)                    [1152, L]
  logits[p, l] = (P[:,p]*10*ma[p]) . (P[:,l]*mm[l]/max(|P[:,l]|,1e-4))
  sm = softmax over l (free dim)            [p, l]
  Yt = max(sm * post[l,p], 1e-8),  post = (1+0.5*mask_c)*mm[l]*ma[p]
  col[p, :] = Yt @ RW,  RW = im2col4x4s2(x1_pad)  [L, 2048]
  y = col2im(col)/4 ; out = concat_g relu(dilated_conv3x3(y, fuse_w[g]) + fuse_b[g])

Sharding: 8 cores = 2 batches x 4 chunks of 576 p-columns (padded to 640).
Kernel 1 (per core): Gram matmul -> softmax -> post-mul -> PE transpose ->
second matmul -> col chunk. Kernel 2 (per core = batch x row-quarter):
4 dilated fuse convs on a 40-row halo slab. Host does im2col / col2im /
scaling prep (pure indexing + tiny elementwise only).
"""
import numpy as np
import ml_dtypes
from contextlib import ExitStack

import concourse.bass as bass
import concourse.bacc as bacc
import concourse.tile as tile
import concourse.mybir as mybir
from concourse import bass_utils
from concourse.bass import ts
from concourse.masks import make_identity

BF16 = mybir.dt.bfloat16
F32 = mybir.dt.float32
H = W = 48
L = H * W           # 2304
C = 128
CHUNK = 576         # L/4 p-columns per core
CHUNKP = 640        # padded to 5*128
SCALE = 10.0
DILS = (1, 2, 4, 8)
NT = [512, 512, 512, 512, 256]   # l-dim tiling of 2304

_cache = {}


# ---------------------------------------------------------------- host prep
def _im2col3(x):
    # x [C,H,W] -> [C*9, H*W] with zero pad 1 (c-major, then ki, kj)
    Cc, Hh, Ww = x.shape
    xp = np.pad(x, ((0, 0), (1, 1), (1, 1)))
    cols = np.empty((Cc, 3, 3, Hh, Ww), np.float32)
    for ki in range(3):
        for kj in range(3):
            cols[:, ki, kj] = xp[:, ki:ki + Hh, kj:kj + Ww]
    return cols.reshape(Cc * 9, Hh * Ww)


def _im2col4s2(x):
    # x [C,96,96] -> [L, C*16], k=4 stride 2 pad 1
    Cc = x.shape[0]
    xp = np.pad(x, ((0, 0), (1, 1), (1, 1)))
    out = np.empty((H, W, Cc, 4, 4), np.float32)
    for ki in range(4):
        for kj in range(4):
            out[:, :, :, ki, kj] = xp[:, ki:ki + 2 * H:2, kj:kj + 2 * W:2].transpose(1, 2, 0)
    return out.reshape(L, Cc * 16)


def _neighbor_mask():
    M = np.zeros((L, L), np.float32)
    p = np.arange(L)
    pi, pj = p // W, p % W
    for off, sel in ((-1, pj >= 1), (1, pj <= W - 2), (W, pi <= H - 2), (-W, pi >= 1)):
        M[p[sel] + off, p[sel]] = 1.0
    return M


def _col2im(col):
    # col [L, C*16] -> [C, 96, 96] scatter-add (stride 2, pad 1)
    colr = col.reshape(H, W, C, 4, 4)
    out = np.zeros((C, 99, 99), np.float32)
    for ki in range(4):
        for kj in range(4):
            out[:, ki:ki + 96:2, kj:kj + 96:2] += colr[:, :, :, ki, kj].transpose(2, 0, 1)
    return out[:, 1:97, 1:97]


def _pack_part(a, p):
    # [N, F] -> [p, N//p, F] partition-major packing (row r = t*p + pp)
    n, f = a.shape
    return np.ascontiguousarray(a.reshape(n // p, p, f).transpose(1, 0, 2))


# ---------------------------------------------------------------- kernels
def _build_main():
    nc = bacc.Bacc("TRN2", target_bir_lowering=False, debug=False, num_devices=8)
    lhs = nc.dram_tensor("lhsP", [128, 9, CHUNKP], BF16, kind="ExternalInput").ap()
    rhs = nc.dram_tensor("rhsP", [128, 9, L], BF16, kind="ExternalInput").ap()
    post = nc.dram_tensor("post", [128, 5, L], BF16, kind="ExternalInput").ap()
    rw = nc.dram_tensor("rw", [128, 18, 2048], BF16, kind="ExternalInput").ap()
    col = nc.dram_tensor("col", [128, 5, 2048], F32, kind="ExternalOutput").ap()

    with tile.TileContext(nc) as tc, ExitStack() as ctx:
        const = ctx.enter_context(tc.tile_pool(name="const", bufs=1))
        ident = const.tile([128, 128], BF16)
        make_identity(nc, ident)
        ins = ctx.enter_context(tc.tile_pool(name="ins", bufs=1))
        s_lhs = ins.tile([128, 9, CHUNKP], BF16, tag="lhs")
        s_rhs = ins.tile([128, 9, L], BF16, tag="rhs")
        s_post = ins.tile([128, 5, L], BF16, tag="post")
        s_rw = ins.tile([128, 18, 2048], BF16, tag="rw")
        # split DMAs so the first matmuls' deps land early (lhs m=0 first)
        for m in range(5):
            nc.sync.dma_start(s_lhs[:, :, ts(m, 128)], lhs[:, :, ts(m, 128)])
        for k in range(9):  # first l-slice per k-tile: first matmul starts early
            nc.sync.dma_start(s_rhs[:, k, 0:512], rhs[:, k, 0:512])
        off = 512
        for sz in NT[1:]:
            nc.sync.dma_start(s_rhs[:, :, off:off + sz], rhs[:, :, off:off + sz])
            off += sz
        for m in range(5):
            nc.sync.dma_start(s_post[:, m, :], post[:, m, :])
        for nn in range(4):
            nc.sync.dma_start(s_rw[:, :, ts(nn, 512)], rw[:, :, ts(nn, 512)])

        # PSUM banks: Ysl 5 + P2 1 + pT 2 = 8
        psum1 = ctx.enter_context(tc.tile_pool(name="psum1", bufs=5, space="PSUM"))
        psum2 = ctx.enter_context(tc.tile_pool(name="psum2", bufs=2, space="PSUM"))
        psumT = ctx.enter_context(tc.tile_pool(name="psumT", bufs=1, space="PSUM"))
        work = ctx.enter_context(tc.tile_pool(name="work", bufs=2))
        stats = ctx.enter_context(tc.tile_pool(name="stats", bufs=3))

        def mm1_stage(m):
            ysl = []
            mx = stats.tile([128, 5], F32, tag="mx")
            for n, sz in enumerate(NT):
                off = n * 512
                Y = psum1.tile([128, 512], F32, tag="Y")
                for k in range(9):
                    nc.tensor.matmul(
                        Y[:, :sz],
                        lhsT=s_lhs[:, k, ts(m, 128)],
                        rhs=s_rhs[:, k, off:off + sz],
                        start=(k == 0), stop=(k == 8),
                    )
                # eager per-slice max (DVE runs under following matmuls)
                nc.vector.reduce_max(mx[:, n:n + 1], Y[:, :sz],
                                     axis=mybir.AxisListType.X)
                ysl.append(Y)
            return ysl, mx

        def rest_stage(m, ysl, mx):
            negm = stats.tile([128, 1], F32, tag="negm")
            nc.vector.tensor_reduce(negm, mx, axis=mybir.AxisListType.X,
                                    op=mybir.AluOpType.max, negate=True)
            sums = stats.tile([128, 5], F32, tag="sums")
            yt1 = work.tile([128, L], BF16, tag="yt1")
            for n, sz in enumerate(NT):
                off = n * 512
                ye = work.tile([128, 512], BF16, tag="ye")
                nc.scalar.activation(ye[:, :sz], ysl[n][:, :sz],
                                     mybir.ActivationFunctionType.Exp,
                                     bias=negm, accum_out=sums[:, n:n + 1])
                nc.vector.tensor_mul(yt1[:, off:off + sz], ye[:, :sz],
                                     s_post[:, m, off:off + sz])
            stot = stats.tile([128, 1], F32, tag="stot")
            nc.vector.reduce_sum(stot, sums, axis=mybir.AxisListType.X)
            rcp = stats.tile([128, 1], F32, tag="rcp")
            nc.vector.reciprocal(rcp, stot)
            yt = work.tile([128, L], BF16, tag="yt")
            nc.vector.tensor_scalar(yt, yt1, scalar1=rcp, scalar2=1e-8,
                                    op0=mybir.AluOpType.mult,
                                    op1=mybir.AluOpType.max)
            # transpose 18 [128,128] blocks: yt [p, l] -> ytT [l, p]
            # batch 4 transposes per PSUM tile -> one evict copy each
            ytT = work.tile([128, 18, 128], BF16, tag="ytT")
            for t0 in range(0, 18, 4):
                nb = min(4, 18 - t0)
                pT = psumT.tile([128, 4, 128], BF16, tag="pT")
                for k in range(t0, t0 + nb):
                    nc.tensor.transpose(pT[:, k - t0, :], yt[:, ts(k, 128)], ident)
                nc.any.tensor_copy(ytT[:, t0:t0 + nb, :], pT[:, :nb, :])
            # second matmul: col[p, co] = sum_l Yt[l, p] * RW[l, co]
            colm = work.tile([128, 2048], F32, tag="colm")
            for nn in range(4):
                P2 = psum2.tile([128, 512], F32, tag="P2")
                for k in range(18):
                    nc.tensor.matmul(P2, lhsT=ytT[:, k, :],
                                     rhs=s_rw[:, k, ts(nn, 512)],
                                     start=(k == 0), stop=(k == 17))
                nc.any.tensor_copy(colm[:, ts(nn, 512)], P2)
                nc.sync.dma_start(col[:, m, ts(nn, 512)], colm[:, ts(nn, 512)])

        # software pipeline: mm1(m+1) overlaps softmax/transpose/mm2 of m
        prev = None
        for m in range(5):
            cur = mm1_stage(m)
            if prev is not None:
                rest_stage(m - 1, *prev)
            prev = cur
        rest_stage(4, *prev)
    nc.compile()
    return nc


def _build_fuse():
    nc = bacc.Bacc("TRN2", target_bir_lowering=False, debug=False, num_devices=8)
    y = nc.dram_tensor("yslab", [128, 40, 112], BF16, kind="ExternalInput").ap()
    fw = nc.dram_tensor("fw", [128, 4, 9, 16], BF16, kind="ExternalInput").ap()
    fb = nc.dram_tensor("fb", [16, 4], F32, kind="ExternalInput").ap()
    fo = nc.dram_tensor("fo", [16, 4, 24 * 96], F32, kind="ExternalOutput").ap()

    RT = [(0, 5), (5, 5), (10, 5), (15, 5), (20, 4)]
    with tile.TileContext(nc) as tc, ExitStack() as ctx:
        ins = ctx.enter_context(tc.tile_pool(name="ins", bufs=1))
        s_w = ins.tile([128, 4, 9, 16], BF16, tag="w")
        nc.sync.dma_start(s_w, fw)
        s_b = ins.tile([16, 4], F32, tag="b")
        nc.sync.dma_start(s_b, fb)
        s_y = ins.tile([128, 40, 112], BF16, tag="y")
        for rc in range(5):
            nc.sync.dma_start(s_y[:, 8 * rc:8 * rc + 8, :], y[:, 8 * rc:8 * rc + 8, :])
        psum = ctx.enter_context(tc.tile_pool(name="psum", bufs=8, space="PSUM"))
        work = ctx.enter_context(tc.tile_pool(name="work", bufs=8))
        for g in range(4):
            d = DILS[g]
            for r0, nr in RT:
                ps = psum.tile([16, 512], F32, tag="ps")
                n = nr * 96
                first = True
                for ki in range(3):
                    for kj in range(3):
                        u0 = 8 + r0 + d * (ki - 1)
                        v0 = 8 + d * (kj - 1)
                        nc.tensor.matmul(
                            ps[:, :n],
                            lhsT=s_w[:, g, ki * 3 + kj, :],
                            rhs=s_y[:, u0:u0 + nr, v0:v0 + 96],
                            start=first, stop=(ki == 2 and kj == 2),
                        )
                        first = False
                ob = work.tile([16, 512], F32, tag="ob")
                nc.scalar.activation(ob[:, :n], ps[:, :n],
                                     mybir.ActivationFunctionType.Relu,
                                     bias=s_b[:, g:g + 1])
                nc.sync.dma_start(fo[:, g, r0 * 96:r0 * 96 + n], ob[:, :n])
    nc.compile()
    return nc


def _get(name, builder):
    if name not in _cache:
        _cache[name] = builder()
    return _cache[name]


# ---------------------------------------------------------------- entry
def kernel(x1, x2, mask, mask_all, fuse_w, fuse_b, _collect=None):
    x1 = np.asarray(x1, np.float32)
    x2 = np.asarray(x2, np.float32)
    mask = np.asarray(mask, np.float32)
    mask_all = np.asarray(mask_all, np.float32)
    fuse_w = np.asarray(fuse_w, np.float32)
    fuse_b = np.asarray(fuse_b, np.float32)
    N = x1.shape[0]
    bf = ml_dtypes.bfloat16

    NB = _neighbor_mask()  # [L, L]
    in_maps = []
    for b in range(N):
        P = _im2col3(x2[b])                       # [1152, L]
        norms = np.sqrt((P * P).sum(0))
        mp = _im2col3(mask[b])                    # [9, L]
        mm = (mp.mean(0) == 0.0).astype(np.float32)
        ma = mask_all[b, 0].reshape(L)
        lhs_full = P * (SCALE * ma)[None, :]      # scale col p
        rhs_full = P * (mm / np.maximum(norms, 1e-4))[None, :]
        rhs_r = _pack_part(rhs_full, 128).astype(bf)      # [128, 9, L]
        RW = _im2col4s2(x1[b])                    # [L, 2048]
        rw_r = _pack_part(RW, 128).astype(bf)             # [128, 18, 2048]
        postF = (1.0 + 0.5 * NB) * mm[:, None] * ma[None, :]  # [l, p]
        for j in range(4):
            sl = slice(j * CHUNK, (j + 1) * CHUNK)
            lhs_c = np.zeros((1152, CHUNKP), np.float32)
            lhs_c[:, :CHUNK] = lhs_full[:, sl]
            post_c = np.zeros((CHUNKP, L), np.float32)
            post_c[:CHUNK] = postF.T[sl]          # [p, l]
            in_maps.append({
                "lhsP": _pack_part(lhs_c, 128).astype(bf),
                "rhsP": rhs_r,
                "post": _pack_part(post_c, 128).astype(bf),
                "rw": rw_r,
            })
    nc1 = _get("main", _build_main)
    res1 = bass_utils.run_bass_kernel_spmd(nc1, in_maps, core_ids=list(range(8)))
    if _collect is not None:
        _collect.append(res1)

    ys = []
    for b in range(N):
        cols = []
        for j in range(4):
            r = res1.results[b * 4 + j]["col"]     # [128, 5, 2048]
            cols.append(r.transpose(1, 0, 2).reshape(CHUNKP, 2048)[:CHUNK])
        col = np.concatenate(cols, 0)              # [L, 2048]
        ys.append(_col2im(col) / 4.0)
    y = np.stack(ys)                               # [N, 128, 96, 96]

    fw_r = np.ascontiguousarray(
        fuse_w.transpose(2, 0, 3, 4, 1).reshape(128, 4, 9, 16)).astype(bf)
    fb_r = np.ascontiguousarray(fuse_b.T).astype(np.float32)  # [16, 4]
    in_maps2 = []
    for b in range(N):
        yp = np.pad(y[b], ((0, 0), (8, 8), (8, 8))).astype(bf)  # [128,112,112]
        for q in range(4):
            in_maps2.append({
                "yslab": np.ascontiguousarray(yp[:, 24 * q:24 * q + 40, :]),
                "fw": fw_r, "fb": fb_r,
            })
    nc2 = _get("fuse", _build_fuse)
    res2 = bass_utils.run_bass_kernel_spmd(nc2, in_maps2, core_ids=list(range(8)))
    if _collect is not None:
        _collect.append(res2)

    out = np.empty((N, 64, 96, 96), np.float32)
    for b in range(N):
        for q in range(4):
            r = res2.results[b * 4 + q]            # [16, 4, 2304]
            o = r["fo"].reshape(16, 4, 24, 96)
            out[b, :, 24 * q:24 * q + 24, :] = o.transpose(1, 0, 2, 3).reshape(64, 24, 96)
    return out
